# revision 1
# baseline (speedup 1.0000x reference)
"""Trainium2 Bass kernel for nn_AutoregressiveDecoder (8-core data parallel).

Strategy:
  - Pure data parallel: B=16384 rows sharded 2048/core across 8 NeuronCores.
  - MLP compute runs feature-major (features on partitions, batch on the free
    dim) in bf16 so weights act as the matmul stationary operand.
  - seq_embed @ w1[:512] is step-invariant -> computed once per 512-row
    macro-tile ("base"), per-step only the small state/onehot extra columns
    are matmul'd and added.
  - seq ships host-pre-transposed [D, BL] so seqT loads are contiguous.
  - Per-row scalar plumbing (losses, bce, clips, selects, state scatter) runs
    in a "blocked" batch-major layout [32 partitions, 16 blocks x 32 slots],
    bridged to/from feature-major with single-instruction 32x32
    StreamTransposes on the vector engine.
  - Index-only preprocessing (ALL_PERMS lookup, one-hot, take_along_axis
    gathers) happens host-side in numpy; loss partial sums are reduced
    host-side (psum of 4 scalars x 8 cores).
"""

import numpy as np
import ml_dtypes

import concourse.bass as bass
import concourse.bacc as bacc
import concourse.tile as tile
from concourse import mybir
from concourse.bass_utils import run_bass_kernel_spmd

BF16 = mybir.dt.bfloat16
F32 = mybir.dt.float32
AF = mybir.ActivationFunctionType
ALU = mybir.AluOpType
NP_BF16 = ml_dtypes.bfloat16

B, D, H = 16384, 512, 512
NCORES = 8
NB = 512            # macro-tile rows (matmul free dim)
ALL_PERMS = np.array(
    [[0, 1, 2], [0, 2, 1], [1, 0, 2], [1, 2, 0], [2, 0, 1], [2, 1, 0]], np.int32
)

# blocked-layout slot map (32 slots per 32-row block)
S_P, S_FL, S_ROH, S_F, S_E = 0, 3, 6, 9, 12   # each 3 wide


def r3(t, s):
    """view a [32, 16*s] tile as [32 p, 16 j, s slots]"""
    return t[:, :].rearrange("p (j s) -> p j s", s=s)


def _enable_ldw_opt():
    """walrus --enable-ldw-opt=false is hardcoded; flip it (dedups LDWEIGHTS)."""
    from concourse import bass_utils as bu
    if getattr(bu, "_ldw_patched", False):
        return
    orig = bu.run_command

    def patched(cmd, *a, **k):
        cmd = list(cmd)  # ldw-opt=true crashes walrus on this BIR; keep off
        return orig(cmd, *a, **k)

    bu.run_command = patched
    bu._ldw_patched = True


def build_graph(BL):
    """Build the per-core Bass graph. BL = rows per core (multiple of NB)."""
    _enable_ldw_opt()
    NM = BL // NB          # macro-tiles per core
    NBLK = NB // 32        # 32-row blocks per macro-tile (16)
    BLKT = BL // 32        # total blocks per core

    nc = bacc.Bacc("TRN2", target_bir_lowering=False, debug=False,
                   num_devices=NCORES)

    # ---- dram parameters -------------------------------------------------
    U8 = mybir.dt.uint8
    seq_d = nc.dram_tensor("seq", [D, BL], BF16, kind="ExternalInput").ap()
    gts_d = nc.dram_tensor("gts", [96, BLKT * 3], F32, kind="ExternalInput").ap()
    roh_d = nc.dram_tensor("roh", [96, BLKT * 3], F32, kind="ExternalInput").ap()
    # uint8 copies of the masks (CopyPredicated wants integer predicates)
    mi_d = nc.dram_tensor("mi", [96, BLKT], U8, kind="ExternalInput").ap()
    rohi_d = nc.dram_tensor("rohi", [96, BLKT * 3], U8, kind="ExternalInput").ap()

    pw1_d = nc.dram_tensor("pw1", [D, H], BF16, kind="ExternalInput").ap()
    pw1x_d = nc.dram_tensor("pw1x", [9, H], BF16, kind="ExternalInput").ap()
    pb1_d = nc.dram_tensor("pb1", [H], F32, kind="ExternalInput").ap()
    pw2_d = nc.dram_tensor("pw2", [H, H // 2], BF16, kind="ExternalInput").ap()
    pb2_d = nc.dram_tensor("pb2", [H // 2], F32, kind="ExternalInput").ap()
    pw3_d = nc.dram_tensor("pw3p", [H // 2, 32], BF16, kind="ExternalInput").ap()

    fw1_d = nc.dram_tensor("fw1", [D, H], BF16, kind="ExternalInput").ap()
    fw1x_d = nc.dram_tensor("fw1x", [15, H], BF16, kind="ExternalInput").ap()
    fb1_d = nc.dram_tensor("fb1", [H], F32, kind="ExternalInput").ap()
    fw2_d = nc.dram_tensor("fw2", [H, H], BF16, kind="ExternalInput").ap()
    fb2_d = nc.dram_tensor("fb2", [H], F32, kind="ExternalInput").ap()
    fw3_d = nc.dram_tensor("fw3p", [H, 32], BF16, kind="ExternalInput").ap()
    eye_d = nc.dram_tensor("eye", [128, 128], BF16, kind="ExternalInput").ap()
    b3s_d = nc.dram_tensor("b3s", [1, 3], F32, kind="ExternalInput").ap()

    df_d = nc.dram_tensor("df", [BL, 3], F32, kind="ExternalOutput").ap()
    dp_d = nc.dram_tensor("dp", [BL, 3], F32, kind="ExternalOutput").ap()
    de_d = nc.dram_tensor("de", [BL, 3], F32, kind="ExternalOutput").ap()
    # raw per-step MLP outputs (loss terms are finished host-side)
    lg_d = nc.dram_tensor("lgo", [BL, 3], F32, kind="ExternalOutput").ap()
    pf_d = nc.dram_tensor("pfo", [BL, 3], F32, kind="ExternalOutput").ap()
    pe_d = nc.dram_tensor("peo", [BL, 3], F32, kind="ExternalOutput").ap()

    v = nc.vector
    sc = nc.scalar
    te = nc.tensor

    with tile.TileContext(nc) as tc:
        wpool = tc.alloc_tile_pool(name="w", bufs=1)
        pers = tc.alloc_tile_pool(name="pers", bufs=1)
        bigp = tc.alloc_tile_pool(name="big", bufs=3)
        stp = tc.alloc_tile_pool(name="stp", bufs=2)
        smp = tc.alloc_tile_pool(name="smp", bufs=4)
        psp = tc.alloc_tile_pool(name="psum", bufs=6, space="PSUM")
        ps3 = tc.alloc_tile_pool(name="psum3", bufs=2, space="PSUM")

        # ---- load weights (persistent) ----------------------------------
        pw1_sb = wpool.tile([128, 4 * H], BF16)
        fw1_sb = wpool.tile([128, 4 * H], BF16)
        pw2_sb = wpool.tile([128, 4 * (H // 2)], BF16)
        fw2_sb = wpool.tile([128, 4 * H], BF16)
        for k in range(4):
            nc.scalar.dma_start(pw1_sb[:, k * H:(k + 1) * H],
                                pw1_d[k * 128:(k + 1) * 128, :])
            nc.scalar.dma_start(fw1_sb[:, k * H:(k + 1) * H],
                                fw1_d[k * 128:(k + 1) * 128, :])
        # biases as [128, nchunk] f32 (partition p, chunk m) for ACT bias APs
        pb1_sb = wpool.tile([128, 4], F32)
        fb1_sb = wpool.tile([128, 4], F32)
        fb2_sb = wpool.tile([128, 4], F32)
        pb2_sb = wpool.tile([128, 2], F32)
        nc.scalar.dma_start(pb1_sb[:, :],
                            pb1_d.rearrange("(m p) -> p m", p=128))
        nc.scalar.dma_start(fb1_sb[:, :],
                            fb1_d.rearrange("(m p) -> p m", p=128))
        for k in range(4):
            nc.scalar.dma_start(pw2_sb[:, k * 256:(k + 1) * 256],
                                pw2_d[k * 128:(k + 1) * 128, :])
            nc.scalar.dma_start(fw2_sb[:, k * H:(k + 1) * H],
                                fw2_d[k * 128:(k + 1) * 128, :])
        nc.scalar.dma_start(fb2_sb[:, :],
                            fb2_d.rearrange("(m p) -> p m", p=128))
        nc.scalar.dma_start(pb2_sb[:, :],
                            pb2_d.rearrange("(m p) -> p m", p=128))
        pw1x_sb = wpool.tile([9, H], BF16)
        fw1x_sb = wpool.tile([47, H], BF16)
        nc.scalar.dma_start(pw1x_sb[:, :], pw1x_d[:, :])
        nc.scalar.dma_start(fw1x_sb[32:47, :], fw1x_d[:, :])
        pw3_sb = wpool.tile([128, 64], BF16)   # 2 k-chunks x 32 cols
        fw3_sb = wpool.tile([128, 128], BF16)  # 4 k-chunks x 32 cols
        for k in range(2):
            nc.scalar.dma_start(pw3_sb[:, k * 32:(k + 1) * 32],
                                pw3_d[k * 128:(k + 1) * 128, :])
        for k in range(4):
            nc.scalar.dma_start(fw3_sb[:, k * 32:(k + 1) * 32],
                                fw3_d[k * 128:(k + 1) * 128, :])
        eye_sb = wpool.tile([128, 128], BF16)
        nc.scalar.dma_start(eye_sb[:, :], eye_d[:, :])
        # l3 output biases broadcast to 32 partitions: cols (pb3, fb3_0, fb3_1)
        b3s_sb = wpool.tile([1, 3], F32)
        nc.scalar.dma_start(b3s_sb[:, :], b3s_d[:, :])
        b3bc = wpool.tile([32, 3], F32)
        nc.gpsimd.partition_broadcast(b3bc[:, :], b3s_sb[:, :])

        # ---- phase 1: all macro-tile bases (PE-dense, keeps HAM warm) ---
        base_p, base_f, sts, lgos, pfos, peos = {}, {}, {}, {}, {}, {}
        for mt in range(NM):
            rows = slice(mt * NB, (mt + 1) * NB)
            seqT = []
            for k in range(4):
                t = stp.tile([128, NB], BF16, tag=f"seqT{k}")
                nc.sync.dma_start(t[:, :], seq_d[k * 128:(k + 1) * 128, rows])
                seqT.append(t)
            base_p[mt] = pers.tile([128, 4 * NB], BF16, tag=f"base_p{mt}", name=f"base_p{mt}")
            base_f[mt] = pers.tile([128, 4 * NB], BF16, tag=f"base_f{mt}", name=f"base_f{mt}")
            for m in range(4):
                pp = psp.tile([128, NB], F32, tag="ps_main")
                for k in range(4):
                    te.matmul(pp[:, :],
                              pw1_sb[:, k * H + m * 128: k * H + (m + 1) * 128],
                              seqT[k][:, :], start=(k == 0), stop=(k == 3))
                sc.activation(base_p[mt][:, m * NB:(m + 1) * NB], pp[:, :],
                              AF.Identity, bias=pb1_sb[:, m:m + 1])
                pf_ = psp.tile([128, NB], F32, tag="ps_main")
                for k in range(4):
                    te.matmul(pf_[:, :],
                              fw1_sb[:, k * H + m * 128: k * H + (m + 1) * 128],
                              seqT[k][:, :], start=(k == 0), stop=(k == 3))
                sc.activation(base_f[mt][:, m * NB:(m + 1) * NB], pf_[:, :],
                              AF.Identity, bias=fb1_sb[:, m:m + 1])
            sts[mt] = pers.tile([32, NBLK * 32], F32, tag=f"st{mt}", name=f"st{mt}")
            v.memset(sts[mt][:, :], 0.0)
            lgos[mt] = pers.tile([32, NBLK * 3], F32, tag=f"lgo{mt}", name=f"lgo{mt}")
            pfos[mt] = pers.tile([32, NBLK * 3], F32, tag=f"pfo{mt}", name=f"pfo{mt}")
            peos[mt] = pers.tile([32, NBLK * 3], F32, tag=f"peo{mt}", name=f"peo{mt}")

        # ---- phase 2: autoregressive steps, macro-interleaved -----------
        # While macro mt's plumbing runs on DVE/ACT, the PE executes the
        # other macros' matmuls (engines are in-order; interleaved emission
        # is what lets the scheduler fill the gaps).
        for s in range(3):
            for mt in range(NM):
                st3 = r3(sts[mt], 32)
                gt_sb = smp.tile([32, NBLK * 3], F32, tag="gt")
                nc.sync.dma_start(
                    gt_sb[:, :],
                    gts_d[s * 32:(s + 1) * 32,
                          mt * NBLK * 3:(mt + 1) * NBLK * 3])
                gt3 = r3(gt_sb, 3)
                gtf, gtp, gte = gt3[:, :, 0:1], gt3[:, :, 1:2], gt3[:, :, 2:3]
                mi_sb = smp.tile([32, NBLK], U8, tag="mi")
                nc.sync.dma_start(
                    mi_sb[:, :],
                    mi_d[s * 32:(s + 1) * 32, mt * NBLK:(mt + 1) * NBLK])
                msk = r3(mi_sb, 1)[:, :, 0:1]
                roi_sb = smp.tile([32, NBLK * 3], U8, tag="roi")
                nc.sync.dma_start(
                    roi_sb[:, :],
                    rohi_d[s * 32:(s + 1) * 32,
                           mt * NBLK * 3:(mt + 1) * NBLK * 3])
                roi3 = r3(roi_sb, 3)
                roh_sb = smp.tile([32, NBLK * 3], F32, tag="rohs")
                nc.sync.dma_start(
                    roh_sb[:, :],
                    roh_d[s * 32:(s + 1) * 32,
                          mt * NBLK * 3:(mt + 1) * NBLK * 3])
                roh3 = r3(roh_sb, 3)

                # bridge to feature-major: cast + stream-transpose
                # (st slots 6..8 stay zero; the one-hot joins here)
                st_bf = smp.tile([32, NBLK * 32], BF16, tag="stbf")
                v.tensor_copy(st_bf[:, :], sts[mt][:, :])
                v.tensor_copy(r3(st_bf, 32)[:, :, S_ROH:S_ROH + 3],
                              roh3[:, :, :])
                exT = smp.tile([64, NBLK * 32], BF16, tag="exT")
                v.transpose(exT[0:32, :], st_bf[:, :])
                v.tensor_copy(exT[32:47, :], exT[0:15, :])

                # layer 1: base + extra @ w1x -> gelu -> h1 (bf16)
                h1p = bigp.tile([128, 4 * NB], BF16, tag="h1p")
                h1f = bigp.tile([128, 4 * NB], BF16, tag="h1f")
                for m in range(4):
                    pp = psp.tile([128, NB], F32, tag="ps_main")
                    pf_ = psp.tile([128, NB], F32, tag="ps_main")
                    te.matmul(pp[:, :], pw1x_sb[:, m * 128:(m + 1) * 128],
                              exT[0:9, :], start=True, stop=False)
                    te.matmul(pf_[:, :],
                              fw1x_sb[32:47, m * 128:(m + 1) * 128],
                              exT[32:47, :], start=True, stop=False,
                              tile_position=(32, 0))
                    te.matmul(pp[:, :], eye_sb[:, :],
                              base_p[mt][:, m * NB:(m + 1) * NB],
                              start=False, stop=True)
                    te.matmul(pf_[:, :], eye_sb[:, :],
                              base_f[mt][:, m * NB:(m + 1) * NB],
                              start=False, stop=True)
                    sc.activation(h1p[:, m * NB:(m + 1) * NB], pp[:, :],
                                  AF.Gelu)
                    sc.activation(h1f[:, m * NB:(m + 1) * NB], pf_[:, :],
                                  AF.Gelu)

                # layer 2
                h2p = bigp.tile([128, 2 * NB], BF16, tag="h2p")
                for m in range(2):
                    pp = psp.tile([128, NB], F32, tag="ps_main")
                    for k in range(4):
                        te.matmul(pp[:, :],
                                  pw2_sb[:, k * 256 + m * 128:
                                         k * 256 + (m + 1) * 128],
                                  h1p[:, k * NB:(k + 1) * NB],
                                  start=(k == 0), stop=(k == 3))
                    sc.activation(h2p[:, m * NB:(m + 1) * NB], pp[:, :],
                                  AF.Gelu, bias=pb2_sb[:, m:m + 1])
                h2f = bigp.tile([128, 4 * NB], BF16, tag="h2f")
                for m in range(4):
                    pp = psp.tile([128, NB], F32, tag="ps_main")
                    for k in range(4):
                        te.matmul(pp[:, :],
                                  fw2_sb[:, k * H + m * 128:
                                         k * H + (m + 1) * 128],
                                  h1f[:, k * NB:(k + 1) * NB],
                                  start=(k == 0), stop=(k == 3))
                    sc.activation(h2f[:, m * NB:(m + 1) * NB], pp[:, :],
                                  AF.Gelu, bias=fb2_sb[:, m:m + 1])

                # layer 3 (padded to M=32; pres in col-group 0, fe in
                # col-group 1 of the same psum tile -> MMs run concurrently)
                p3 = ps3.tile([64, NB], F32, tag="ps3")
                for k in range(2):
                    te.matmul(p3[0:32, :], pw3_sb[:, k * 32:(k + 1) * 32],
                              h2p[:, k * NB:(k + 1) * NB],
                              start=(k == 0), stop=(k == 1))
                for k in range(4):
                    te.matmul(p3[32:64, :], fw3_sb[:, k * 32:(k + 1) * 32],
                              h2f[:, k * NB:(k + 1) * NB],
                              start=(k == 0), stop=(k == 3),
                              tile_position=(0, 32))

                # bridge back to blocked batch-major; add b3 biases after
                lgT = smp.tile([32, NBLK * 32], F32, tag="lgT")
                v.transpose(lgT[:, :], p3[0:32, :])
                feT = smp.tile([32, NBLK * 32], F32, tag="feT")
                v.transpose(feT[:, :], p3[32:64, :])
                lg3 = r3(lgT, 32)
                fe3 = r3(feT, 32)
                logit = lg3[:, :, 0:1]
                pf = fe3[:, :, 0:1]
                pe = fe3[:, :, 1:2]
                v.tensor_scalar_add(logit, logit, b3bc[:, 0:1])
                v.tensor_scalar_add(pf, pf, b3bc[:, 1:2])
                v.tensor_scalar_add(pe, pe, b3bc[:, 2:3])

                # ---- plumbing (all [32, NBLK, 1] APs) -------------------
                pb = smp.tile([32, NBLK * 8], F32, tag="pb")
                pb3d = r3(pb, 8)
                a_f, a_p, a_e = pb3d[:, :, 0:1], pb3d[:, :, 1:2], pb3d[:, :, 2:3]
                sig, pfc, pec = pb3d[:, :, 3:4], pb3d[:, :, 4:5], pb3d[:, :, 5:6]

                # raw outputs for host-side loss
                v.tensor_copy(r3(lgos[mt], 3)[:, :, s:s + 1], logit)
                v.tensor_copy(r3(pfos[mt], 3)[:, :, s:s + 1], pf)
                v.tensor_copy(r3(peos[mt], 3)[:, :, s:s + 1], pe)

                # sigmoid(l) = 0.5*tanh(0.5*l) + 0.5 (stays on the gelu table)
                sc.activation(sig, logit, AF.Tanh, scale=0.5)
                v.tensor_scalar(sig, sig, 0.5, 0.5, ALU.mult, ALU.add)
                v.tensor_scalar(pfc, pf, -10.0, 10.0, ALU.max, ALU.min)
                v.tensor_scalar(pec, pe, -100.0, 100.0, ALU.max, ALU.min)
                v.tensor_copy(a_f, gtf)
                v.copy_predicated(a_f, msk, pfc)
                v.tensor_copy(a_p, gtp)
                v.copy_predicated(a_p, msk, sig)
                v.tensor_copy(a_e, gte)
                v.copy_predicated(a_e, msk, pec)

                # state scatter: slot = act where roh_r else keep
                for r in range(3):
                    rp = roi3[:, :, r:r + 1]
                    v.copy_predicated(st3[:, :, S_P + r:S_P + r + 1], rp, a_p)
                    v.tensor_max(st3[:, :, S_FL + r:S_FL + r + 1],
                                 st3[:, :, S_FL + r:S_FL + r + 1],
                                 roh3[:, :, r:r + 1])
                    v.copy_predicated(st3[:, :, S_F + r:S_F + r + 1], rp, a_f)
                    v.copy_predicated(st3[:, :, S_E + r:S_E + r + 1], rp, a_e)

                if s == 2:
                    rows = slice(mt * NB, (mt + 1) * NB)
                    nc.gpsimd.dma_start(
                        df_d[rows, :].rearrange("(j p) r -> p j r", p=32),
                        st3[:, :, S_F:S_F + 3])
                    nc.gpsimd.dma_start(
                        dp_d[rows, :].rearrange("(j p) r -> p j r", p=32),
                        st3[:, :, S_P:S_P + 3])
                    nc.gpsimd.dma_start(
                        de_d[rows, :].rearrange("(j p) r -> p j r", p=32),
                        st3[:, :, S_E:S_E + 3])
                    nc.gpsimd.dma_start(
                        lg_d[rows, :].rearrange("(j p) r -> p j r", p=32),
                        r3(lgos[mt], 3)[:, :, :])
                    nc.gpsimd.dma_start(
                        pf_d[rows, :].rearrange("(j p) r -> p j r", p=32),
                        r3(pfos[mt], 3)[:, :, :])
                    nc.gpsimd.dma_start(
                        pe_d[rows, :].rearrange("(j p) r -> p j r", p=32),
                        r3(peos[mt], 3)[:, :, :])

        # ---- outputs (unreachable marker) -------------------------------
        for mt in []:
            rows = slice(mt * NB, (mt + 1) * NB)
            st3 = r3(sts[mt], 32)
            nc.gpsimd.dma_start(
                df_d[rows, :].rearrange("(j p) r -> p j r", p=32),
                st3[:, :, S_F:S_F + 3])
            nc.gpsimd.dma_start(
                dp_d[rows, :].rearrange("(j p) r -> p j r", p=32),
                st3[:, :, S_P:S_P + 3])
            nc.gpsimd.dma_start(
                de_d[rows, :].rearrange("(j p) r -> p j r", p=32),
                st3[:, :, S_E:S_E + 3])
            nc.gpsimd.dma_start(
                lg_d[rows, :].rearrange("(j p) r -> p j r", p=32),
                r3(lgos[mt], 3)[:, :, :])
            nc.gpsimd.dma_start(
                pf_d[rows, :].rearrange("(j p) r -> p j r", p=32),
                r3(pfos[mt], 3)[:, :, :])
            nc.gpsimd.dma_start(
                pe_d[rows, :].rearrange("(j p) r -> p j r", p=32),
                r3(peos[mt], 3)[:, :, :])

        for p in (ps3, psp, smp, stp, bigp, pers, wpool):
            p.release()

    nc.compile()
    return nc


# ---------------------------------------------------------------------------
def prep_inputs(seq_embed, freq, pres, enrich,
                pw1, pb1, pw2, pb2, pw3, pb3,
                fw1, fb1, fw2, fb2, fw3, fb3,
                perm_idx, round_mask, BL):
    """Host-side (numpy) sharding + index preprocessing."""
    f32 = np.float32
    seq = np.asarray(seq_embed, f32)
    perms = ALL_PERMS[np.asarray(perm_idx)]                    # [B,3]
    gtf = np.take_along_axis(np.asarray(freq, f32), perms, 1)   # [B,3] (col=s)
    gtp = np.take_along_axis(np.asarray(pres, f32), perms, 1)
    gte = np.take_along_axis(np.asarray(enrich, f32), perms, 1)
    m = np.take_along_axis(np.asarray(round_mask), perms, 1).astype(f32)
    roh = (perms[:, :, None] == np.arange(3)[None, None, :]).astype(f32)  # [B,3s,3r]

    bf = lambda a: np.ascontiguousarray(np.asarray(a, f32).astype(NP_BF16))
    pw1x = bf(np.asarray(pw1, f32)[512:521][[0, 2, 4, 1, 3, 5, 6, 7, 8]])
    fw1x = bf(np.asarray(fw1, f32)[512:527][
        [1, 5, 9, 3, 7, 11, 12, 13, 14, 0, 4, 8, 2, 6, 10]])
    pw3p = np.zeros((256, 32), f32); pw3p[:, 0] = np.asarray(pw3, f32)[:, 0]
    fw3p = np.zeros((512, 32), f32); fw3p[:, 0:2] = np.asarray(fw3, f32)
    b3s = np.array([[np.asarray(pb3, f32)[0],
                     np.asarray(fb3, f32)[0], np.asarray(fb3, f32)[1]]], f32)

    shared = {
        "pw1": bf(np.asarray(pw1, f32)[:512]), "pw1x": pw1x,
        "pb1": np.ascontiguousarray(np.asarray(pb1, f32)),
        "pw2": bf(pw2), "pb2": np.ascontiguousarray(np.asarray(pb2, f32)),
        "pw3p": bf(pw3p),
        "fw1": bf(np.asarray(fw1, f32)[:512]), "fw1x": fw1x,
        "fb1": np.ascontiguousarray(np.asarray(fb1, f32)),
        "fw2": bf(fw2), "fb2": np.ascontiguousarray(np.asarray(fb2, f32)),
        "fw3p": bf(fw3p),
        "eye": np.eye(128, dtype=NP_BF16),
        "b3s": b3s,
    }

    in_maps = []
    ncores = seq.shape[0] // BL
    BLKT = BL // 32
    for c in range(ncores):
        rs = slice(c * BL, (c + 1) * BL)
        # blocked layouts: index [s*32+p, Jg*w + q], b_local = 32*Jg + p
        gt3 = np.stack([gtf[rs], gtp[rs], gte[rs]], -1)          # [BL,3s,3]
        gt3 = gt3.reshape(BLKT, 32, 3, 3).transpose(2, 1, 0, 3)  # [3s,32,J,3]
        rohc = roh[rs].reshape(BLKT, 32, 3, 3).transpose(2, 1, 0, 3)
        mc = m[rs].reshape(BLKT, 32, 3).transpose(2, 1, 0)       # [3s,32,J]
        in_maps.append(dict(
            seq=np.ascontiguousarray(seq[rs].astype(NP_BF16).T),
            gts=np.ascontiguousarray(gt3.reshape(3 * 32, BLKT * 3)),
            roh=np.ascontiguousarray(rohc.reshape(3 * 32, BLKT * 3)),
            mi=np.ascontiguousarray(mc.reshape(3 * 32, BLKT).astype(np.uint8)),
            rohi=np.ascontiguousarray(
                rohc.reshape(3 * 32, BLKT * 3).astype(np.uint8)),
            **shared))
    aux = dict(gtf=gtf, gtp=gtp, gte=gte, m=m)
    return in_maps, aux


def assemble(results, aux):
    """Gather per-core outputs; finish the (tiny) loss reductions host-side."""
    f32 = np.float32
    df = np.concatenate([r["df"] for r in results], 0).astype(f32)
    dp = np.concatenate([r["dp"] for r in results], 0).astype(f32)
    de = np.concatenate([r["de"] for r in results], 0).astype(f32)
    lg = np.concatenate([r["lgo"] for r in results], 0).astype(f32)
    pf = np.concatenate([r["pfo"] for r in results], 0).astype(f32)
    pe = np.concatenate([r["peo"] for r in results], 0).astype(f32)
    m, gtf, gtp, gte = aux["m"], aux["gtf"], aux["gtp"], aux["gte"]
    lf = np.sum(np.square(pf - gtf) * m, dtype=np.float64)
    le = np.sum(np.square(pe - gte) * m, dtype=np.float64)
    bce = (np.maximum(lg, 0.0) - lg * gtp
           + np.log1p(np.exp(-np.abs(lg), dtype=np.float64)))
    lp = np.sum(bce * m, dtype=np.float64)
    nm = np.sum(m, dtype=np.float64) + 1e-8
    head = np.array([lf / nm, lp / nm, le / nm], f32)
    return np.concatenate([head, df.ravel(), dp.ravel(), de.ravel()])


_CACHE = {}


def _get_graph(BL):
    if BL not in _CACHE:
        _CACHE[BL] = build_graph(BL)
    return _CACHE[BL]


def _install_profile_hook():
    """Provide antenv.axon_hooks (missing in this image) so trace=True works."""
    import sys, types
    try:
        import antenv.axon_hooks  # noqa: F401
        return
    except ImportError:
        pass
    from trn_agent_boot.trn_boot import _ntff_profile_via_ctypes
    hook = _ntff_profile_via_ctypes('/opt/axon/libaxon_pjrt.so')
    mod = types.ModuleType('antenv.axon_hooks')
    mod._hook = hook
    mod.get_axon_ntff_profile_hook = lambda: mod._hook
    mod.set_axon_ntff_profile_hook = lambda h: setattr(mod, '_hook', h)
    sys.modules['antenv.axon_hooks'] = mod


def run(inputs, trace=False):
    if trace:
        _install_profile_hook()
    BL = inputs["seq_embed"].shape[0] // NCORES
    nc = _get_graph(BL)
    in_maps, aux = prep_inputs(**inputs, BL=BL)
    res = run_bass_kernel_spmd(nc, in_maps, core_ids=list(range(NCORES)),
                               trace=trace)
    out = assemble(res.results, aux)
    return out, res


def kernel(**inputs):
    inputs = {k: np.asarray(v) for k, v in inputs.items()}
    out, _ = run(inputs)
    return out



# revision 7
# speedup vs baseline: 1.1304x; 1.1304x over previous
"""Trainium2 Bass kernel for nn_AutoregressiveDecoder (8-core data parallel).

Strategy (v2):
  - Pure data parallel: B=16384 rows sharded 2048/core across 8 NeuronCores.
  - MLP compute runs feature-major (features on partitions, batch on the free
    dim) so weights act as the matmul stationary operand.
  - seq_embed @ w1[:512] is step-invariant -> computed once per 512-row
    macro-tile ("base") in fp8 DoubleRow, per-step only the small state/onehot
    extra columns are matmul'd (bf16) and the base re-added via an eye matmul.
  - L1 (phase 1), L2 and L3 matmuls run fp8-e4m3 with DoubleRow perf mode
    (K=256 per instruction); h1/h2 activations are written as fp8 by the
    gelu ACT pass directly.
  - b1 rides in the one-hot rows of W1x (they sum to 1 every step), so the
    phase-1 psum->sbuf move is a bias-free DVE copy, not an ACT pass.
  - h1 gelu processes (pres, fe) psum pairs in one FD=1024 instruction.
  - Per-row scalar plumbing runs in a blocked batch-major layout
    [32 partitions, 16 blocks x 32 slots] bridged with 32x32 DVE
    StreamTransposes; state uses r-major (F,P,E) triplets so masked scatters
    and selects batch 3 slots per instruction with broadcast masks.
  - Index-only preprocessing (ALL_PERMS lookup, one-hot, gathers) happens
    host-side in numpy; loss partial sums are reduced host-side.
"""

import numpy as np
import ml_dtypes

import concourse.bass as bass
import concourse.bacc as bacc
import concourse.tile as tile
from concourse import mybir
from concourse.bass_utils import run_bass_kernel_spmd

BF16 = mybir.dt.bfloat16
F32 = mybir.dt.float32
F8 = mybir.dt.float8e4
AF = mybir.ActivationFunctionType
ALU = mybir.AluOpType
DR = mybir.MatmulPerfMode.DoubleRow
NP_BF16 = ml_dtypes.bfloat16
NP_F8 = ml_dtypes.float8_e4m3

B, D, H = 16384, 512, 512
NCORES = 8
NB = 512            # macro-tile rows (matmul free dim)
ALL_PERMS = np.array(
    [[0, 1, 2], [0, 2, 1], [1, 0, 2], [1, 2, 0], [2, 0, 1], [2, 1, 0]], np.int32
)

# state-tile slot map (32 slots per 32-row block):
#   slots 0-8:  (F_r, P_r, E_r) r-major triplets
#   slots 9-11: decoded flags FL_r
S_TRIP, S_FL = 0, 9
# st_bf / exT feature-major row order (what W1x contracts against):
#   rows 0-8:  P(3), FL(3), ROH(3)   <- pres_net input tail
#   rows 9-14: F(3), E(3)            <- rest of fe_net input tail
# pb scratch slots: 0-2 (a_f, a_p, a_e), 3-5 (pfc, sig, pec)


def r3(t, s):
    """view a [32, 16*s] tile as [32 p, 16 j, s slots]"""
    return t[:, :].rearrange("p (j s) -> p j s", s=s)


def trip(t, k):
    """r-major triplet view: [32, 16 j, 3 r] AP over slots {k, k+3, k+6}."""
    return (
        r3(t, 32)[:, :, 0:9]
        .rearrange("p j (r q) -> p j r q", q=3)[:, :, :, k]
    )


def build_graph(BL):
    """Build the per-core Bass graph. BL = rows per core (multiple of NB)."""
    NM = BL // NB          # macro-tiles per core
    NBLK = NB // 32        # 32-row blocks per macro-tile (16)
    BLKT = BL // 32        # total blocks per core

    nc = bacc.Bacc("TRN2", target_bir_lowering=False, debug=False,
                   num_devices=NCORES)

    # ---- dram parameters -------------------------------------------------
    U8 = mybir.dt.uint8
    seq_d = nc.dram_tensor("seq", [D, BL], F8, kind="ExternalInput").ap()
    gts_d = nc.dram_tensor("gts", [96, BLKT * 3], F32, kind="ExternalInput").ap()
    roh_d = nc.dram_tensor("roh", [96, BLKT * 3], F32, kind="ExternalInput").ap()
    # uint8 copies of the masks (CopyPredicated wants integer predicates)
    mi_d = nc.dram_tensor("mi", [96, BLKT], U8, kind="ExternalInput").ap()
    rohi_d = nc.dram_tensor("rohi", [96, BLKT * 3], U8, kind="ExternalInput").ap()

    pw1_d = nc.dram_tensor("pw1dr", [128, 2048], F8, kind="ExternalInput").ap()
    fw1_d = nc.dram_tensor("fw1dr", [128, 2048], F8, kind="ExternalInput").ap()
    pw1x_d = nc.dram_tensor("pw1x", [9, H], BF16, kind="ExternalInput").ap()
    fw1x_d = nc.dram_tensor("fw1x", [15, H], BF16, kind="ExternalInput").ap()
    pw2_d = nc.dram_tensor("pw2dr", [128, 1024], F8, kind="ExternalInput").ap()
    fw2_d = nc.dram_tensor("fw2dr", [128, 2048], F8, kind="ExternalInput").ap()
    pb2_d = nc.dram_tensor("pb2", [H // 2], F32, kind="ExternalInput").ap()
    fb2_d = nc.dram_tensor("fb2", [H], F32, kind="ExternalInput").ap()
    pw3_d = nc.dram_tensor("pw3dr", [128, 64], F8, kind="ExternalInput").ap()
    fw3_d = nc.dram_tensor("fw3dr", [128, 128], F8, kind="ExternalInput").ap()
    eye_d = nc.dram_tensor("eye", [128, 128], BF16, kind="ExternalInput").ap()
    b3s_d = nc.dram_tensor("b3s", [1, 3], F32, kind="ExternalInput").ap()

    df_d = nc.dram_tensor("df", [BL, 3], F32, kind="ExternalOutput").ap()
    dp_d = nc.dram_tensor("dp", [BL, 3], F32, kind="ExternalOutput").ap()
    de_d = nc.dram_tensor("de", [BL, 3], F32, kind="ExternalOutput").ap()
    # raw per-step MLP outputs (loss terms are finished host-side)
    lg_d = nc.dram_tensor("lgo", [BL, 3], F32, kind="ExternalOutput").ap()
    pf_d = nc.dram_tensor("pfo", [BL, 3], F32, kind="ExternalOutput").ap()
    pe_d = nc.dram_tensor("peo", [BL, 3], F32, kind="ExternalOutput").ap()

    v = nc.vector
    sc = nc.scalar
    te = nc.tensor

    with tile.TileContext(nc) as tc:
        wpool = tc.alloc_tile_pool(name="w", bufs=1)
        pers = tc.alloc_tile_pool(name="pers", bufs=1)
        bigp = tc.alloc_tile_pool(name="big", bufs=3)
        stp = tc.alloc_tile_pool(name="stp", bufs=2)
        smp = tc.alloc_tile_pool(name="smp", bufs=4)
        pp = tc.alloc_tile_pool(name="pspair", bufs=2, space="PSUM")
        ps1 = tc.alloc_tile_pool(name="pssing", bufs=2, space="PSUM")
        ps3 = tc.alloc_tile_pool(name="psum3", bufs=2, space="PSUM")

        # ---- load weights (persistent) ----------------------------------
        pw1_sb = wpool.tile([128, 2048], F8)
        fw1_sb = wpool.tile([128, 2048], F8)
        pw2_sb = wpool.tile([128, 1024], F8)
        fw2_sb = wpool.tile([128, 2048], F8)
        nc.sync.dma_start(pw1_sb[:, :], pw1_d[:, :])
        nc.sync.dma_start(fw1_sb[:, :], fw1_d[:, :])
        nc.sync.dma_start(pw2_sb[:, :], pw2_d[:, :])
        nc.sync.dma_start(fw2_sb[:, :], fw2_d[:, :])
        pb2_sb = wpool.tile([128, 2], F32)
        fb2_sb = wpool.tile([128, 4], F32)
        nc.sync.dma_start(pb2_sb[:, :],
                          pb2_d.rearrange("(m p) -> p m", p=128))
        nc.sync.dma_start(fb2_sb[:, :],
                          fb2_d.rearrange("(m p) -> p m", p=128))
        pw1x_sb = wpool.tile([9, H], BF16)
        fw1x_sb = wpool.tile([47, H], BF16)
        nc.sync.dma_start(pw1x_sb[:, :], pw1x_d[:, :])
        nc.sync.dma_start(fw1x_sb[32:47, :], fw1x_d[:, :])
        pw3_sb = wpool.tile([128, 64], F8)
        fw3_sb = wpool.tile([128, 128], F8)
        nc.sync.dma_start(pw3_sb[:, :], pw3_d[:, :])
        nc.sync.dma_start(fw3_sb[:, :], fw3_d[:, :])
        eye_sb = wpool.tile([128, 128], BF16)
        nc.sync.dma_start(eye_sb[:, :], eye_d[:, :])
        # l3 output biases broadcast to 32 partitions: cols (pb3, fb3_0, fb3_1)
        b3s_sb = wpool.tile([1, 3], F32)
        nc.sync.dma_start(b3s_sb[:, :], b3s_d[:, :])
        b3bc = wpool.tile([32, 3], F32)
        nc.gpsimd.partition_broadcast(b3bc[:, :], b3s_sb[:, :])

        def w1s(t, a, c):          # phase-1 w1 slice [128, 2, 128]
            off = (a * 4 + c) * 256
            return t[:, off:off + 256].rearrange("p (j m) -> p j m", m=128)

        def pw2s(a, c):
            off = (a * 2 + c) * 256
            return pw2_sb[:, off:off + 256].rearrange("p (j m) -> p j m", m=128)

        def fw2s(a, c):
            off = (a * 4 + c) * 256
            return fw2_sb[:, off:off + 256].rearrange("p (j m) -> p j m", m=128)

        # ---- phase 1: all macro-tile bases (fp8 DoubleRow) --------------
        base_p, base_f, sts, stbf, lgos, pfos, peos = {}, {}, {}, {}, {}, {}, {}
        for mt in range(NM):
            rows = slice(mt * NB, (mt + 1) * NB)
            seqT = stp.tile([128, 4 * NB], F8, tag="seqT")
            for k in range(4):
                nc.sync.dma_start(seqT[:, k * NB:(k + 1) * NB],
                                  seq_d[k * 128:(k + 1) * 128, rows])
            seqr = seqT[:, :].rearrange("p (j n) -> p j n", n=NB)
            base_p[mt] = pers.tile([128, 4 * NB], BF16, tag=f"base_p{mt}",
                                   name=f"base_p{mt}")
            base_f[mt] = pers.tile([128, 4 * NB], BF16, tag=f"base_f{mt}",
                                   name=f"base_f{mt}")
            for pair in range(2):
                psa = pp.tile([128, 2 * NB], F32, tag="ps_pair")
                for ci in range(2):
                    c = 2 * pair + ci
                    for a in range(2):
                        te.matmul(psa[:, ci * NB:(ci + 1) * NB],
                                  w1s(pw1_sb, a, c), seqr[:, 2 * a:2 * a + 2, :],
                                  start=(a == 0), stop=(a == 1), perf_mode=DR)
                v.tensor_copy(base_p[mt][:, pair * 1024:(pair + 1) * 1024],
                              psa[:, :])
                psb = pp.tile([128, 2 * NB], F32, tag="ps_pair")
                for ci in range(2):
                    c = 2 * pair + ci
                    for a in range(2):
                        te.matmul(psb[:, ci * NB:(ci + 1) * NB],
                                  w1s(fw1_sb, a, c), seqr[:, 2 * a:2 * a + 2, :],
                                  start=(a == 0), stop=(a == 1), perf_mode=DR)
                v.tensor_copy(base_f[mt][:, pair * 1024:(pair + 1) * 1024],
                              psb[:, :])
            sts[mt] = pers.tile([32, NBLK * 32], F32, tag=f"st{mt}", name=f"st{mt}")
            v.memset(sts[mt][:, :], 0.0)
            stbf[mt] = pers.tile([32, NBLK * 32], BF16, tag=f"stbf{mt}",
                                 name=f"stbf{mt}")
            v.memset(stbf[mt][:, :], 0.0)
            lgos[mt] = pers.tile([32, NBLK * 3], F32, tag=f"lgo{mt}", name=f"lgo{mt}")
            pfos[mt] = pers.tile([32, NBLK * 3], F32, tag=f"pfo{mt}", name=f"pfo{mt}")
            peos[mt] = pers.tile([32, NBLK * 3], F32, tag=f"peo{mt}", name=f"peo{mt}")

        # ---- phase 2: autoregressive steps, macro-interleaved -----------
        # While macro mt's plumbing runs on DVE/ACT, the PE executes the
        # other macros' matmuls (engines are in-order; interleaved emission
        # is what lets the scheduler fill the gaps).
        for s in range(3):
            for mt in range(NM):
                stv = r3(sts[mt], 32)
                sbv = r3(stbf[mt], 32)
                gt_sb = smp.tile([32, NBLK * 3], F32, tag="gt")
                nc.sync.dma_start(
                    gt_sb[:, :],
                    gts_d[s * 32:(s + 1) * 32,
                          mt * NBLK * 3:(mt + 1) * NBLK * 3])
                gt3 = r3(gt_sb, 3)
                mi_sb = smp.tile([32, NBLK], U8, tag="mi")
                nc.sync.dma_start(
                    mi_sb[:, :],
                    mi_d[s * 32:(s + 1) * 32, mt * NBLK:(mt + 1) * NBLK])
                roi_sb = smp.tile([32, NBLK * 3], U8, tag="roi")
                nc.sync.dma_start(
                    roi_sb[:, :],
                    rohi_d[s * 32:(s + 1) * 32,
                           mt * NBLK * 3:(mt + 1) * NBLK * 3])
                roi3 = r3(roi_sb, 3)
                roh_sb = smp.tile([32, NBLK * 3], F32, tag="rohs")
                nc.sync.dma_start(
                    roh_sb[:, :],
                    roh_d[s * 32:(s + 1) * 32,
                          mt * NBLK * 3:(mt + 1) * NBLK * 3])
                roh3 = r3(roh_sb, 3)

                # bridge to feature-major: reorder r-major state triplets
                # into the exT feature order, then stream-transpose
                if s > 0:
                    v.tensor_copy(sbv[:, :, 0:3], trip(sts[mt], 1))   # P
                    v.tensor_copy(sbv[:, :, 3:6], stv[:, :, 9:12])    # FL
                    v.tensor_copy(sbv[:, :, 9:12], trip(sts[mt], 0))  # F
                    v.tensor_copy(sbv[:, :, 12:15], trip(sts[mt], 2)) # E
                v.tensor_copy(sbv[:, :, 6:9], roh3[:, :, :])          # ROH
                exT = smp.tile([64, NBLK * 32], BF16, tag="exT")
                v.transpose(exT[0:32, :], stbf[mt][:, :])
                v.tensor_copy(exT[32:47, :], exT[0:15, :])

                # layer 1: extra @ w1x (bf16) + eye-add of base -> gelu(fp8)
                h1 = bigp.tile([128, 8 * NB], F8, tag="h1")
                h1v = h1[:, :].rearrange("p (net m n) -> p m net n",
                                         net=2, n=NB)
                for m in range(4):
                    xps = pp.tile([128, 2 * NB], F32, tag="ps_pair")
                    te.matmul(xps[:, 0:NB],
                              pw1x_sb[:, m * 128:(m + 1) * 128],
                              exT[0:9, :], start=True, stop=False)
                    te.matmul(xps[:, NB:2 * NB],
                              fw1x_sb[32:47, m * 128:(m + 1) * 128],
                              exT[32:47, :], start=True, stop=False,
                              tile_position=(32, 0))
                    te.matmul(xps[:, 0:NB], eye_sb[:, :],
                              base_p[mt][:, m * NB:(m + 1) * NB],
                              start=False, stop=True)
                    te.matmul(xps[:, NB:2 * NB], eye_sb[:, :],
                              base_f[mt][:, m * NB:(m + 1) * NB],
                              start=False, stop=True)
                    sc.activation(h1v[:, m:m + 1].rearrange(
                                      "p m net n -> p (m net) n"),
                                  xps[:, :].rearrange("p (net n) -> p net n",
                                                      n=NB),
                                  AF.Gelu)

                h1r = h1[:, :].rearrange("p (j n) -> p j n", n=NB)
                # layer 2 (fp8 DoubleRow) -> gelu(fp8) with b2 bias
                h2 = bigp.tile([128, 6 * NB], F8, tag="h2")
                for c in range(2):
                    ps2 = ps1.tile([128, NB], F32, tag="ps2")
                    for a in range(2):
                        te.matmul(ps2[:, :], pw2s(a, c),
                                  h1r[:, 2 * a:2 * a + 2, :],
                                  start=(a == 0), stop=(a == 1), perf_mode=DR)
                    sc.activation(h2[:, c * NB:(c + 1) * NB], ps2[:, :],
                                  AF.Gelu, bias=pb2_sb[:, c:c + 1])
                for c in range(4):
                    ps2 = ps1.tile([128, NB], F32, tag="ps2")
                    for a in range(2):
                        te.matmul(ps2[:, :], fw2s(a, c),
                                  h1r[:, 4 + 2 * a:6 + 2 * a, :],
                                  start=(a == 0), stop=(a == 1), perf_mode=DR)
                    sc.activation(h2[:, (2 + c) * NB:(3 + c) * NB], ps2[:, :],
                                  AF.Gelu, bias=fb2_sb[:, c:c + 1])

                # layer 3 (plain fp8, padded to M=32; DoubleRow is rejected
                # for <128-partition psum dsts.  pres in col-group 0, fe in
                # col-group 1 of the same psum tile -> MMs run concurrently)
                h2r = h2[:, :].rearrange("p (j n) -> p j n", n=NB)
                p3 = ps3.tile([64, NB], F32, tag="ps3")
                for kk in range(2):
                    te.matmul(p3[0:32, :], pw3_sb[:, kk * 32:(kk + 1) * 32],
                              h2r[:, kk], start=(kk == 0), stop=(kk == 1))
                for kk in range(4):
                    te.matmul(p3[32:64, :], fw3_sb[:, kk * 32:(kk + 1) * 32],
                              h2r[:, 2 + kk], start=(kk == 0), stop=(kk == 3),
                              tile_position=(0, 32))

                # bridge back to blocked batch-major; add b3 biases after
                lgT = smp.tile([32, NBLK * 32], F32, tag="lgT")
                v.transpose(lgT[:, :], p3[0:32, :])
                feT = smp.tile([32, NBLK * 32], F32, tag="feT")
                v.transpose(feT[:, :], p3[32:64, :])
                lg3 = r3(lgT, 32)
                fe3 = r3(feT, 32)
                logit = lg3[:, :, 0:1]
                pf = fe3[:, :, 0:1]
                pe = fe3[:, :, 1:2]
                v.tensor_scalar_add(logit, logit, b3bc[:, 0:1])
                v.tensor_scalar_add(pf, pf, b3bc[:, 1:2])
                v.tensor_scalar_add(pe, pe, b3bc[:, 2:3])

                # ---- plumbing (all [32, NBLK, k] APs) -------------------
                pb = smp.tile([32, NBLK * 8], F32, tag="pb")
                pb3d = r3(pb, 8)
                a_all = pb3d[:, :, 0:3]
                pfc, sig, pec = (pb3d[:, :, 3:4], pb3d[:, :, 4:5],
                                 pb3d[:, :, 5:6])

                # raw outputs for host-side loss
                v.tensor_copy(r3(lgos[mt], 3)[:, :, s:s + 1], logit)
                v.tensor_copy(r3(pfos[mt], 3)[:, :, s:s + 1], pf)
                v.tensor_copy(r3(peos[mt], 3)[:, :, s:s + 1], pe)

                # sigmoid(l) = 0.5*tanh(0.5*l) + 0.5 (stays on the gelu table)
                sc.activation(sig, logit, AF.Tanh, scale=0.5)
                v.tensor_scalar(sig, sig, 0.5, 0.5, ALU.mult, ALU.add)
                v.tensor_scalar(pfc, pf, -10.0, 10.0, ALU.max, ALU.min)
                v.tensor_scalar(pec, pe, -100.0, 100.0, ALU.max, ALU.min)
                # a = mask ? clipped-prediction : ground-truth  (f, p, e)
                mib = r3(mi_sb, 1)[:, :, 0:1].broadcast_to((32, NBLK, 3))
                v.select(a_all, mib, pb3d[:, :, 3:6], gt3[:, :, 0:3])

                # state scatter: triplet r = act where roh_r else keep
                for r in range(3):
                    rp3 = roi3[:, :, r:r + 1].broadcast_to((32, NBLK, 3))
                    v.copy_predicated(stv[:, :, 3 * r:3 * r + 3], rp3, a_all)
                    v.tensor_max(stv[:, :, 9 + r:10 + r],
                                 stv[:, :, 9 + r:10 + r],
                                 roh3[:, :, r:r + 1])

                if s == 2:
                    rows = slice(mt * NB, (mt + 1) * NB)
                    # repack stride-3 triplets into contiguous staging tiles
                    # (DMA needs a contiguous inner dim)
                    dout = smp.tile([32, NBLK * 9], F32, tag="dout")
                    do3 = dout[:, :].rearrange("p (k j r) -> p k j r",
                                               k=3, r=3)
                    for k in range(3):
                        v.tensor_copy(do3[:, k], trip(sts[mt], k))
                    nc.gpsimd.dma_start(
                        df_d[rows, :].rearrange("(j p) r -> p j r", p=32),
                        do3[:, 0])
                    nc.gpsimd.dma_start(
                        dp_d[rows, :].rearrange("(j p) r -> p j r", p=32),
                        do3[:, 1])
                    nc.gpsimd.dma_start(
                        de_d[rows, :].rearrange("(j p) r -> p j r", p=32),
                        do3[:, 2])
                    nc.gpsimd.dma_start(
                        lg_d[rows, :].rearrange("(j p) r -> p j r", p=32),
                        r3(lgos[mt], 3)[:, :, :])
                    nc.gpsimd.dma_start(
                        pf_d[rows, :].rearrange("(j p) r -> p j r", p=32),
                        r3(pfos[mt], 3)[:, :, :])
                    nc.gpsimd.dma_start(
                        pe_d[rows, :].rearrange("(j p) r -> p j r", p=32),
                        r3(peos[mt], 3)[:, :, :])

        for p in (ps3, ps1, pp, smp, stp, bigp, pers, wpool):
            p.release()

    nc.compile()
    return nc


# ---------------------------------------------------------------------------
def _dr_pack(w, mc):
    """Pack [K, M] weights into the DoubleRow stationary layout
    [128, (K//256) * (M//mc) * 2 * mc] with index (a, c, jj, m)."""
    K, M = w.shape
    a, c = K // 256, M // mc
    v = w.reshape(a, 2, 128, c, mc)          # [a, jj, p, c, m]
    v = v.transpose(2, 0, 3, 1, 4)           # [p, a, c, jj, m]
    return np.ascontiguousarray(v.reshape(128, a * c * 2 * mc))


def prep_inputs(seq_embed, freq, pres, enrich,
                pw1, pb1, pw2, pb2, pw3, pb3,
                fw1, fb1, fw2, fb2, fw3, fb3,
                perm_idx, round_mask, BL):
    """Host-side (numpy) sharding + index preprocessing."""
    f32 = np.float32
    seq = np.asarray(seq_embed, f32)
    perms = ALL_PERMS[np.asarray(perm_idx)]                    # [B,3]
    gtf = np.take_along_axis(np.asarray(freq, f32), perms, 1)   # [B,3] (col=s)
    gtp = np.take_along_axis(np.asarray(pres, f32), perms, 1)
    gte = np.take_along_axis(np.asarray(enrich, f32), perms, 1)
    m = np.take_along_axis(np.asarray(round_mask), perms, 1).astype(f32)
    roh = (perms[:, :, None] == np.arange(3)[None, None, :]).astype(f32)  # [B,3s,3r]

    bf = lambda a: np.ascontiguousarray(np.asarray(a, f32).astype(NP_BF16))
    f8 = lambda a: np.ascontiguousarray(np.asarray(a, f32).astype(NP_F8))
    # W1x rows in exT order (P, FL, ROH, F, E); b1 folded into the ROH rows
    pw1x = np.asarray(pw1, f32)[512:521][[0, 2, 4, 1, 3, 5, 6, 7, 8]].copy()
    pw1x[6:9] += np.asarray(pb1, f32)[None, :]
    fw1x = np.asarray(fw1, f32)[512:527][
        [1, 5, 9, 3, 7, 11, 12, 13, 14, 0, 4, 8, 2, 6, 10]].copy()
    fw1x[6:9] += np.asarray(fb1, f32)[None, :]
    pw3p = np.zeros((256, 32), f32); pw3p[:, 0] = np.asarray(pw3, f32)[:, 0]
    fw3p = np.zeros((512, 32), f32); fw3p[:, 0:2] = np.asarray(fw3, f32)
    b3s = np.array([[np.asarray(pb3, f32)[0],
                     np.asarray(fb3, f32)[0], np.asarray(fb3, f32)[1]]], f32)

    shared = {
        "pw1dr": f8(_dr_pack(np.asarray(pw1, f32)[:512], 128)),
        "fw1dr": f8(_dr_pack(np.asarray(fw1, f32)[:512], 128)),
        "pw1x": bf(pw1x), "fw1x": bf(fw1x),
        "pw2dr": f8(_dr_pack(np.asarray(pw2, f32), 128)),
        "fw2dr": f8(_dr_pack(np.asarray(fw2, f32), 128)),
        "pb2": np.ascontiguousarray(np.asarray(pb2, f32)),
        "fb2": np.ascontiguousarray(np.asarray(fb2, f32)),
        "pw3dr": f8(_dr_pack(pw3p, 32)),
        "fw3dr": f8(_dr_pack(fw3p, 32)),
        "eye": np.eye(128, dtype=NP_BF16),
        "b3s": b3s,
    }

    in_maps = []
    ncores = seq.shape[0] // BL
    BLKT = BL // 32
    for c in range(ncores):
        rs = slice(c * BL, (c + 1) * BL)
        # blocked layouts: index [s*32+p, Jg*w + q], b_local = 32*Jg + p
        gt3 = np.stack([gtf[rs], gtp[rs], gte[rs]], -1)          # [BL,3s,3]
        gt3 = gt3.reshape(BLKT, 32, 3, 3).transpose(2, 1, 0, 3)  # [3s,32,J,3]
        rohc = roh[rs].reshape(BLKT, 32, 3, 3).transpose(2, 1, 0, 3)
        mc = m[rs].reshape(BLKT, 32, 3).transpose(2, 1, 0)       # [3s,32,J]
        in_maps.append(dict(
            seq=np.ascontiguousarray(seq[rs].astype(NP_F8).T),
            gts=np.ascontiguousarray(gt3.reshape(3 * 32, BLKT * 3)),
            roh=np.ascontiguousarray(rohc.reshape(3 * 32, BLKT * 3)),
            mi=np.ascontiguousarray(mc.reshape(3 * 32, BLKT).astype(np.uint8)),
            rohi=np.ascontiguousarray(
                rohc.reshape(3 * 32, BLKT * 3).astype(np.uint8)),
            **shared))
    aux = dict(gtf=gtf, gtp=gtp, gte=gte, m=m)
    return in_maps, aux


def assemble(results, aux):
    """Gather per-core outputs; finish the (tiny) loss reductions host-side."""
    f32 = np.float32
    df = np.concatenate([r["df"] for r in results], 0).astype(f32)
    dp = np.concatenate([r["dp"] for r in results], 0).astype(f32)
    de = np.concatenate([r["de"] for r in results], 0).astype(f32)
    lg = np.concatenate([r["lgo"] for r in results], 0).astype(f32)
    pf = np.concatenate([r["pfo"] for r in results], 0).astype(f32)
    pe = np.concatenate([r["peo"] for r in results], 0).astype(f32)
    m, gtf, gtp, gte = aux["m"], aux["gtf"], aux["gtp"], aux["gte"]
    lf = np.sum(np.square(pf - gtf) * m, dtype=np.float64)
    le = np.sum(np.square(pe - gte) * m, dtype=np.float64)
    bce = (np.maximum(lg, 0.0) - lg * gtp
           + np.log1p(np.exp(-np.abs(lg), dtype=np.float64)))
    lp = np.sum(bce * m, dtype=np.float64)
    nm = np.sum(m, dtype=np.float64) + 1e-8
    head = np.array([lf / nm, lp / nm, le / nm], f32)
    return np.concatenate([head, df.ravel(), dp.ravel(), de.ravel()])


_CACHE = {}


def _get_graph(BL):
    if BL not in _CACHE:
        _CACHE[BL] = build_graph(BL)
    return _CACHE[BL]


def _install_profile_hook():
    """Provide antenv.axon_hooks (missing in this image) so trace=True works."""
    import sys, types
    try:
        import antenv.axon_hooks  # noqa: F401
        return
    except ImportError:
        pass
    from trn_agent_boot.trn_boot import _ntff_profile_via_ctypes
    hook = _ntff_profile_via_ctypes('/opt/axon/libaxon_pjrt.so')
    mod = types.ModuleType('antenv.axon_hooks')
    mod._hook = hook
    mod.get_axon_ntff_profile_hook = lambda: mod._hook
    mod.set_axon_ntff_profile_hook = lambda h: setattr(mod, '_hook', h)
    sys.modules['antenv.axon_hooks'] = mod


def run(inputs, trace=False):
    if trace:
        _install_profile_hook()
    BL = inputs["seq_embed"].shape[0] // NCORES
    nc = _get_graph(BL)
    in_maps, aux = prep_inputs(**inputs, BL=BL)
    res = run_bass_kernel_spmd(nc, in_maps, core_ids=list(range(NCORES)),
                               trace=trace)
    out = assemble(res.results, aux)
    return out, res


def kernel(**inputs):
    inputs = {k: np.asarray(v) for k, v in inputs.items()}
    out, _ = run(inputs)
    return out


# revision 12
# speedup vs baseline: 1.2470x; 1.1031x over previous
"""Trainium2 Bass kernel for nn_AutoregressiveDecoder (8-core data parallel).

Strategy (v3):
  - Pure data parallel: B=16384 rows sharded 2048/core across 8 NeuronCores.
  - MLP compute runs feature-major (features on partitions, batch on the free
    dim) so weights act as the matmul stationary operand.
  - seq_embed @ w1[:512] is step-invariant -> computed once per 512-row
    macro-tile ("base", fp8 DoubleRow), stored as fp8 next to a per-step
    "extra features" zone in the same tile.  Each step's full L1 is then ONE
    DoubleRow matmul per 128-unit chunk: K-pair j=0 contracts the 15 extra
    rows against W1x (b1 folded into the one-hot rows), j=1 re-adds the base
    chunk through an identity stationary.
  - L2 runs fp8-e4m3 DoubleRow; h1 is written as fp8 by the gelu ACT pass
    directly (FD=1024 psum pairs, pres|fe).  L3 and h2 stay bf16 (DoubleRow
    is rejected for <128-partition psum dsts and plain fp8 gave no speedup).
  - Per-row scalar plumbing runs in a blocked batch-major layout
    [32 partitions, 16 blocks x 32 slots] bridged with 32x32 DVE
    StreamTransposes; state uses r-major (F,P,E) triplets so masked scatters
    and selects batch 3 slots per instruction with broadcast masks.
  - DMA descriptor generation is minimized (it costs ~0.6us/descriptor of
    sequencer time): step inputs are host-packed into 2 tensors, outputs
    into 2 tensors, seq loads are single 3D-AP descriptors, and issuance is
    spread across the sync/scalar/tensor/gpsimd queues.
  - Index-only preprocessing (ALL_PERMS lookup, one-hot, gathers) happens
    host-side in numpy; loss partial sums are reduced host-side.
"""

import numpy as np
import ml_dtypes

import concourse.bass as bass
import concourse.bacc as bacc
import concourse.tile as tile
from concourse import mybir
from concourse.bass_utils import run_bass_kernel_spmd

BF16 = mybir.dt.bfloat16
F32 = mybir.dt.float32
F8 = mybir.dt.float8e4
AF = mybir.ActivationFunctionType
ALU = mybir.AluOpType
DR = mybir.MatmulPerfMode.DoubleRow
NP_BF16 = ml_dtypes.bfloat16
NP_F8 = ml_dtypes.float8_e4m3

B, D, H = 16384, 512, 512
NCORES = 8
NB = 512            # macro-tile rows (matmul free dim)
ALL_PERMS = np.array(
    [[0, 1, 2], [0, 2, 1], [1, 0, 2], [1, 2, 0], [2, 0, 1], [2, 1, 0]], np.int32
)

# state-tile slot map (32 slots per 32-row block):
#   slots 0-8:  (F_r, P_r, E_r) r-major triplets;  slots 9-11: flags FL_r
# st_bf / exT feature-major row order (what W1x contracts against):
#   rows 0-8: P(3), FL(3), ROH(3);  rows 9-14: F(3), E(3)
# pb scratch slots: 0-2 (a_f, a_p, a_e), 3-5 (pfc, sig, pec)


def r3(t, s):
    """view a [32, 16*s] tile as [32 p, 16 j, s slots]"""
    return t[:, :].rearrange("p (j s) -> p j s", s=s)


def trip(t, k):
    """r-major triplet view: [32, 16 j, 3 r] AP over slots {k, k+3, k+6}."""
    return (
        r3(t, 32)[:, :, 0:9]
        .rearrange("p j (r q) -> p j r q", q=3)[:, :, :, k]
    )


def build_graph(BL):
    """Build the per-core Bass graph. BL = rows per core (multiple of NB)."""
    NM = BL // NB          # macro-tiles per core
    NBLK = NB // 32        # 32-row blocks per macro-tile (16)
    BLKT = BL // 32        # total blocks per core

    nc = bacc.Bacc("TRN2", target_bir_lowering=False, debug=False,
                   num_devices=NCORES)

    # ---- dram parameters -------------------------------------------------
    U8 = mybir.dt.uint8
    seq_d = nc.dram_tensor("seq", [D, BL], F8, kind="ExternalInput").ap()
    # packed per-(step,mt) inputs: [gt(3) | roh(3)] f32 and [mi(1)|rohi(3)] u8
    gr_d = nc.dram_tensor("gr", [96, BLKT * 6], F32, kind="ExternalInput").ap()
    mr_d = nc.dram_tensor("mr", [96, BLKT * 4], U8, kind="ExternalInput").ap()

    pw1_d = nc.dram_tensor("pw1dr", [128, 2048], F8, kind="ExternalInput").ap()
    fw1_d = nc.dram_tensor("fw1dr", [128, 2048], F8, kind="ExternalInput").ap()
    pwe_d = nc.dram_tensor("pwe1", [128, 1024], F8, kind="ExternalInput").ap()
    fwe_d = nc.dram_tensor("fwe1", [128, 1024], F8, kind="ExternalInput").ap()
    pw2_d = nc.dram_tensor("pw2dr", [128, 1024], F8, kind="ExternalInput").ap()
    fw2_d = nc.dram_tensor("fw2dr", [128, 2048], F8, kind="ExternalInput").ap()
    pb2_d = nc.dram_tensor("pb2", [H // 2], F32, kind="ExternalInput").ap()
    fb2_d = nc.dram_tensor("fb2", [H], F32, kind="ExternalInput").ap()
    pw3_d = nc.dram_tensor("pw3b", [128, 64], BF16, kind="ExternalInput").ap()
    fw3_d = nc.dram_tensor("fw3b", [128, 128], BF16, kind="ExternalInput").ap()
    b3s_d = nc.dram_tensor("b3s", [1, 3], F32, kind="ExternalInput").ap()

    # packed outputs: dout = [df|dp|de], lpe = [lg|pf|pe] per row
    do_d = nc.dram_tensor("dout", [BL, 9], F32, kind="ExternalOutput").ap()
    lp_d = nc.dram_tensor("lpe", [BL, 9], F32, kind="ExternalOutput").ap()

    v = nc.vector
    sc = nc.scalar
    te = nc.tensor

    with tile.TileContext(nc) as tc:
        wpool = tc.alloc_tile_pool(name="w", bufs=1)
        pers = tc.alloc_tile_pool(name="pers", bufs=1)
        bigp = tc.alloc_tile_pool(name="big", bufs=3)
        stp = tc.alloc_tile_pool(name="stp", bufs=2)
        smp = tc.alloc_tile_pool(name="smp", bufs=4)
        pp = tc.alloc_tile_pool(name="pspair", bufs=2, space="PSUM")
        ps1 = tc.alloc_tile_pool(name="pssing", bufs=2, space="PSUM")
        ps3 = tc.alloc_tile_pool(name="psum3", bufs=2, space="PSUM")

        # ---- weights needed for phase 1 first (sync queue) --------------
        pw1_sb = wpool.tile([128, 2048], F8)
        fw1_sb = wpool.tile([128, 2048], F8)
        nc.sync.dma_start(pw1_sb[:, :], pw1_d[:, :])
        nc.sync.dma_start(fw1_sb[:, :], fw1_d[:, :])
        # remaining weights on the scalar/tensor queues (issue in parallel)
        pwe_sb = wpool.tile([128, 1024], F8)
        fwe_sb = wpool.tile([128, 1024], F8)
        pw2_sb = wpool.tile([128, 1024], F8)
        fw2_sb = wpool.tile([128, 2048], F8)
        nc.scalar.dma_start(pwe_sb[:, :], pwe_d[:, :])
        nc.scalar.dma_start(fwe_sb[:, :], fwe_d[:, :])
        nc.scalar.dma_start(pw2_sb[:, :], pw2_d[:, :])
        nc.scalar.dma_start(fw2_sb[:, :], fw2_d[:, :])
        pb2_sb = wpool.tile([128, 2], F32)
        fb2_sb = wpool.tile([128, 4], F32)
        nc.scalar.dma_start(pb2_sb[:, :],
                            pb2_d.rearrange("(m p) -> p m", p=128))
        nc.scalar.dma_start(fb2_sb[:, :],
                            fb2_d.rearrange("(m p) -> p m", p=128))
        pw3_sb = wpool.tile([128, 64], BF16)
        fw3_sb = wpool.tile([128, 128], BF16)
        nc.scalar.dma_start(pw3_sb[:, :], pw3_d[:, :])
        nc.scalar.dma_start(fw3_sb[:, :], fw3_d[:, :])
        b3s_sb = wpool.tile([1, 3], F32)
        nc.scalar.dma_start(b3s_sb[:, :], b3s_d[:, :])
        b3bc = wpool.tile([32, 3], F32)
        nc.gpsimd.partition_broadcast(b3bc[:, :], b3s_sb[:, :])

        def w1s(t, a, c):          # phase-1 w1 slice [128, 2, 128]
            off = (a * 4 + c) * 256
            return t[:, off:off + 256].rearrange("p (j m) -> p j m", m=128)

        def wes(t, m):             # L1 extra+eye slice [128, 2, 128]
            return t[:, m * 256:(m + 1) * 256].rearrange(
                "p (j m) -> p j m", m=128)

        def pw2s(a, c):
            off = (a * 2 + c) * 256
            return pw2_sb[:, off:off + 256].rearrange("p (j m) -> p j m", m=128)

        def fw2s(a, c):
            off = (a * 4 + c) * 256
            return fw2_sb[:, off:off + 256].rearrange("p (j m) -> p j m", m=128)

        # ---- phase 1: all macro-tile bases (fp8 DoubleRow) --------------
        # bse_* layout [128, 8*NB] fp8: cols 0:NB = extra-feature zone
        # (rows 0-14 live, 15-127 zeroed), chunk m at cols (1+m)*NB.
        bse_p, bse_f, sts, stbf, lgos = {}, {}, {}, {}, {}
        for mt in range(NM):
            rows = slice(mt * NB, (mt + 1) * NB)
            seqT = stp.tile([128, 4 * NB], F8, tag="seqT")
            nc.sync.dma_start(
                seqT[:, :].rearrange("p (j n) -> p j n", n=NB),
                seq_d[:, rows].rearrange("(j p) n -> p j n", p=128))
            seqr = seqT[:, :].rearrange("p (j n) -> p j n", n=NB)
            bse_p[mt] = pers.tile([128, 8 * NB], F8, tag=f"bse_p{mt}",
                                  name=f"bse_p{mt}")
            bse_f[mt] = pers.tile([128, 8 * NB], F8, tag=f"bse_f{mt}",
                                  name=f"bse_f{mt}")
            v.memset(bse_p[mt][:, 0:NB], 0.0)
            v.memset(bse_f[mt][:, 0:NB], 0.0)
            for pair in range(2):
                psa = pp.tile([128, 2 * NB], F32, tag="ps_pair")
                for ci in range(2):
                    c = 2 * pair + ci
                    for a in range(2):
                        te.matmul(psa[:, ci * NB:(ci + 1) * NB],
                                  w1s(pw1_sb, a, c), seqr[:, 2 * a:2 * a + 2, :],
                                  start=(a == 0), stop=(a == 1), perf_mode=DR)
                v.tensor_copy(
                    bse_p[mt][:, (1 + 2 * pair) * NB:(3 + 2 * pair) * NB],
                    psa[:, :])
                psb = pp.tile([128, 2 * NB], F32, tag="ps_pair")
                for ci in range(2):
                    c = 2 * pair + ci
                    for a in range(2):
                        te.matmul(psb[:, ci * NB:(ci + 1) * NB],
                                  w1s(fw1_sb, a, c), seqr[:, 2 * a:2 * a + 2, :],
                                  start=(a == 0), stop=(a == 1), perf_mode=DR)
                v.tensor_copy(
                    bse_f[mt][:, (1 + 2 * pair) * NB:(3 + 2 * pair) * NB],
                    psb[:, :])
            sts[mt] = pers.tile([32, NBLK * 32], F32, tag=f"st{mt}", name=f"st{mt}")
            v.memset(sts[mt][:, :], 0.0)
            stbf[mt] = pers.tile([32, NBLK * 32], BF16, tag=f"stbf{mt}",
                                 name=f"stbf{mt}")
            v.memset(stbf[mt][:, :], 0.0)
            lgos[mt] = pers.tile([32, NBLK * 9], F32, tag=f"lgo{mt}",
                                 name=f"lgo{mt}")

        # ---- phase 2: autoregressive steps, macro-interleaved -----------
        for s in range(3):
            for mt in range(NM):
                stv = r3(sts[mt], 32)
                sbv = r3(stbf[mt], 32)
                gr_sb = smp.tile([32, NBLK * 6], F32, tag="gr")
                nc.gpsimd.dma_start(
                    gr_sb[:, :],
                    gr_d[s * 32:(s + 1) * 32,
                         mt * NBLK * 6:(mt + 1) * NBLK * 6])
                gt3 = gr_sb[:, 0:NBLK * 3].rearrange("p (j s) -> p j s", s=3)
                roh3 = gr_sb[:, NBLK * 3:NBLK * 6].rearrange(
                    "p (j s) -> p j s", s=3)
                mr_sb = smp.tile([32, NBLK * 4], U8, tag="mr")
                nc.gpsimd.dma_start(
                    mr_sb[:, :],
                    mr_d[s * 32:(s + 1) * 32,
                         mt * NBLK * 4:(mt + 1) * NBLK * 4])
                mi1 = mr_sb[:, 0:NBLK].rearrange("p (j s) -> p j s", s=1)
                roi3 = mr_sb[:, NBLK:NBLK * 4].rearrange(
                    "p (j s) -> p j s", s=3)

                # bridge to feature-major: reorder r-major state triplets
                # into the exT feature order, then stream-transpose
                if s > 0:
                    v.tensor_copy(sbv[:, :, 0:3], trip(sts[mt], 1))   # P
                    v.tensor_copy(sbv[:, :, 3:6], stv[:, :, 9:12])    # FL
                    v.tensor_copy(sbv[:, :, 9:12], trip(sts[mt], 0))  # F
                    v.tensor_copy(sbv[:, :, 12:15], trip(sts[mt], 2)) # E
                v.tensor_copy(sbv[:, :, 6:9], roh3[:, :, :])          # ROH
                exT = smp.tile([32, NBLK * 32], BF16, tag="exT")
                v.transpose(exT[0:32, :], stbf[mt][:, :])
                # refresh the extra-feature zones (bf16 -> fp8)
                v.tensor_copy(bse_p[mt][0:15, 0:NB], exT[0:15, :])
                v.tensor_copy(bse_f[mt][0:15, 0:NB], exT[0:15, :])

                # layer 1: ONE DoubleRow matmul per 128-unit chunk and net:
                # j=0 extras @ W1x, j=1 base re-add via identity
                h1 = bigp.tile([128, 8 * NB], F8, tag="h1")
                h1v = h1[:, :].rearrange("p (net m n) -> p m net n",
                                         net=2, n=NB)
                for m in range(4):
                    w = (m + 1) * NB
                    xps = pp.tile([128, 2 * NB], F32, tag="ps_pair")
                    mvp = bse_p[mt][:, 0:2 * w].rearrange(
                        "p (j n) -> p j n", n=w)[:, :, 0:NB]
                    te.matmul(xps[:, 0:NB], wes(pwe_sb, m), mvp,
                              start=True, stop=True, perf_mode=DR)
                    mvf = bse_f[mt][:, 0:2 * w].rearrange(
                        "p (j n) -> p j n", n=w)[:, :, 0:NB]
                    te.matmul(xps[:, NB:2 * NB], wes(fwe_sb, m), mvf,
                              start=True, stop=True, perf_mode=DR)
                    sc.activation(h1v[:, m:m + 1].rearrange(
                                      "p m net n -> p (m net) n"),
                                  xps[:, :].rearrange("p (net n) -> p net n",
                                                      n=NB),
                                  AF.Gelu)

                h1r = h1[:, :].rearrange("p (j n) -> p j n", n=NB)
                # layer 2 (fp8 DoubleRow) -> gelu(bf16) with b2 bias
                h2 = bigp.tile([128, 6 * NB], BF16, tag="h2")
                for c in range(2):
                    ps2 = ps1.tile([128, NB], F32, tag="ps2")
                    for a in range(2):
                        te.matmul(ps2[:, :], pw2s(a, c),
                                  h1r[:, 2 * a:2 * a + 2, :],
                                  start=(a == 0), stop=(a == 1), perf_mode=DR)
                    sc.activation(h2[:, c * NB:(c + 1) * NB], ps2[:, :],
                                  AF.Gelu, bias=pb2_sb[:, c:c + 1])
                for c in range(4):
                    ps2 = ps1.tile([128, NB], F32, tag="ps2")
                    for a in range(2):
                        te.matmul(ps2[:, :], fw2s(a, c),
                                  h1r[:, 4 + 2 * a:6 + 2 * a, :],
                                  start=(a == 0), stop=(a == 1), perf_mode=DR)
                    sc.activation(h2[:, (2 + c) * NB:(3 + c) * NB], ps2[:, :],
                                  AF.Gelu, bias=fb2_sb[:, c:c + 1])

                # layer 3 (bf16, padded to M=32; pres in col-group 0, fe in
                # col-group 1 of the same psum tile -> MMs run concurrently)
                h2r = h2[:, :].rearrange("p (j n) -> p j n", n=NB)
                p3 = ps3.tile([64, NB], F32, tag="ps3")
                for kk in range(2):
                    te.matmul(p3[0:32, :], pw3_sb[:, kk * 32:(kk + 1) * 32],
                              h2r[:, kk], start=(kk == 0), stop=(kk == 1))
                for kk in range(4):
                    te.matmul(p3[32:64, :], fw3_sb[:, kk * 32:(kk + 1) * 32],
                              h2r[:, 2 + kk], start=(kk == 0), stop=(kk == 3),
                              tile_position=(0, 32))

                # bridge back to blocked batch-major; add b3 biases after
                lgT = smp.tile([32, NBLK * 32], F32, tag="lgT")
                v.transpose(lgT[:, :], p3[0:32, :])
                feT = smp.tile([32, NBLK * 32], F32, tag="feT")
                v.transpose(feT[:, :], p3[32:64, :])
                lg3 = r3(lgT, 32)
                fe3 = r3(feT, 32)
                logit = lg3[:, :, 0:1]
                pf = fe3[:, :, 0:1]
                pe = fe3[:, :, 1:2]
                v.tensor_scalar_add(logit, logit, b3bc[:, 0:1])
                v.tensor_scalar_add(pf, pf, b3bc[:, 1:2])
                v.tensor_scalar_add(pe, pe, b3bc[:, 2:3])

                # ---- plumbing (all [32, NBLK, k] APs) -------------------
                pb = smp.tile([32, NBLK * 8], F32, tag="pb")
                pb3d = r3(pb, 8)
                a_all = pb3d[:, :, 0:3]
                pfc, sig, pec = (pb3d[:, :, 3:4], pb3d[:, :, 4:5],
                                 pb3d[:, :, 5:6])

                # raw outputs for host-side loss: lgos layout (j, kind, s)
                lp4 = lgos[mt][:, :].rearrange("p (j k s) -> p j k s",
                                               k=3, s=3)
                v.tensor_copy(lp4[:, :, 0, s], lg3[:, :, 0])
                v.tensor_copy(lp4[:, :, 1, s], fe3[:, :, 0])
                v.tensor_copy(lp4[:, :, 2, s], fe3[:, :, 1])

                # sigmoid(l) = 0.5*tanh(0.5*l) + 0.5 (stays on the gelu table)
                sc.activation(sig, logit, AF.Tanh, scale=0.5)
                v.tensor_scalar(sig, sig, 0.5, 0.5, ALU.mult, ALU.add)
                v.tensor_scalar(pfc, pf, -10.0, 10.0, ALU.max, ALU.min)
                v.tensor_scalar(pec, pe, -100.0, 100.0, ALU.max, ALU.min)
                # a = mask ? clipped-prediction : ground-truth  (f, p, e)
                mib = mi1[:, :, 0:1].broadcast_to((32, NBLK, 3))
                v.select(a_all, mib, pb3d[:, :, 3:6], gt3[:, :, 0:3])

                # state scatter: triplet r = act where roh_r else keep
                for r in range(3):
                    rp3 = roi3[:, :, r:r + 1].broadcast_to((32, NBLK, 3))
                    v.copy_predicated(stv[:, :, 3 * r:3 * r + 3], rp3, a_all)
                    v.tensor_max(stv[:, :, 9 + r:10 + r],
                                 stv[:, :, 9 + r:10 + r],
                                 roh3[:, :, r:r + 1])

                if s == 2:
                    rows = slice(mt * NB, (mt + 1) * NB)
                    # repack stride-3 triplets into one contiguous staging
                    # tile (DMA needs a contiguous inner dim), single DMA out
                    dout = smp.tile([32, NBLK * 9], F32, tag="dout")
                    do4 = dout[:, :].rearrange("p (j k r) -> p j k r",
                                               k=3, r=3)
                    for k in range(3):
                        v.tensor_copy(do4[:, :, k], trip(sts[mt], k))
                    nc.scalar.dma_start(
                        do_d[rows, :].rearrange("(j p) c -> p j c", p=32),
                        r3(dout, 9))
                    nc.gpsimd.dma_start(
                        lp_d[rows, :].rearrange("(j p) c -> p j c", p=32),
                        r3(lgos[mt], 9))

        for p in (ps3, ps1, pp, smp, stp, bigp, pers, wpool):
            p.release()

    nc.compile()
    return nc


# ---------------------------------------------------------------------------
def _dr_pack(w, mc):
    """Pack [K, M] weights into the DoubleRow stationary layout
    [128, (K//256) * (M//mc) * 2 * mc] with index (a, c, jj, m)."""
    K, M = w.shape
    a, c = K // 256, M // mc
    v = w.reshape(a, 2, 128, c, mc)          # [a, jj, p, c, m]
    v = v.transpose(2, 0, 3, 1, 4)           # [p, a, c, jj, m]
    return np.ascontiguousarray(v.reshape(128, a * c * 2 * mc))


def _we_pack(w1x):
    """Pack the L1 extra weights + identity into the DoubleRow stationary
    layout [128, 4 * 2 * 128] with index (m, j, mc):
    j=0 -> W1x rows (padded to 128), j=1 -> I."""
    nrows = w1x.shape[0]
    out = np.zeros((128, 4, 2, 128), np.float32)
    eye = np.eye(128, dtype=np.float32)
    for m in range(4):
        out[0:nrows, m, 0, :] = w1x[:, m * 128:(m + 1) * 128]
        out[:, m, 1, :] = eye
    return out.reshape(128, 1024)


def prep_inputs(seq_embed, freq, pres, enrich,
                pw1, pb1, pw2, pb2, pw3, pb3,
                fw1, fb1, fw2, fb2, fw3, fb3,
                perm_idx, round_mask, BL):
    """Host-side (numpy) sharding + index preprocessing."""
    f32 = np.float32
    seq = np.asarray(seq_embed, f32)
    perms = ALL_PERMS[np.asarray(perm_idx)]                    # [B,3]
    gtf = np.take_along_axis(np.asarray(freq, f32), perms, 1)   # [B,3] (col=s)
    gtp = np.take_along_axis(np.asarray(pres, f32), perms, 1)
    gte = np.take_along_axis(np.asarray(enrich, f32), perms, 1)
    m = np.take_along_axis(np.asarray(round_mask), perms, 1).astype(f32)
    roh = (perms[:, :, None] == np.arange(3)[None, None, :]).astype(f32)  # [B,3s,3r]

    bf = lambda a: np.ascontiguousarray(np.asarray(a, f32).astype(NP_BF16))
    f8 = lambda a: np.ascontiguousarray(np.asarray(a, f32).astype(NP_F8))
    # W1x rows in exT order (P, FL, ROH, F, E); b1 folded into the ROH rows
    pw1x = np.asarray(pw1, f32)[512:521][[0, 2, 4, 1, 3, 5, 6, 7, 8]].copy()
    pw1x[6:9] += np.asarray(pb1, f32)[None, :]
    fw1x = np.asarray(fw1, f32)[512:527][
        [1, 5, 9, 3, 7, 11, 12, 13, 14, 0, 4, 8, 2, 6, 10]].copy()
    fw1x[6:9] += np.asarray(fb1, f32)[None, :]
    pw3p = np.zeros((256, 32), f32); pw3p[:, 0] = np.asarray(pw3, f32)[:, 0]
    fw3p = np.zeros((512, 32), f32); fw3p[:, 0:2] = np.asarray(fw3, f32)
    b3s = np.array([[np.asarray(pb3, f32)[0],
                     np.asarray(fb3, f32)[0], np.asarray(fb3, f32)[1]]], f32)

    shared = {
        "pw1dr": f8(_dr_pack(np.asarray(pw1, f32)[:512], 128)),
        "fw1dr": f8(_dr_pack(np.asarray(fw1, f32)[:512], 128)),
        "pwe1": f8(_we_pack(pw1x)),
        "fwe1": f8(_we_pack(fw1x)),
        "pw2dr": f8(_dr_pack(np.asarray(pw2, f32), 128)),
        "fw2dr": f8(_dr_pack(np.asarray(fw2, f32), 128)),
        "pb2": np.ascontiguousarray(np.asarray(pb2, f32)),
        "fb2": np.ascontiguousarray(np.asarray(fb2, f32)),
        "pw3b": bf(_dr_pack(pw3p, 32)),
        "fw3b": bf(_dr_pack(fw3p, 32)),
        "b3s": b3s,
    }

    in_maps = []
    ncores = seq.shape[0] // BL
    BLKT = BL // 32
    NBLK = 16
    nmt = BLKT // NBLK
    for c in range(ncores):
        rs = slice(c * BL, (c + 1) * BL)
        # blocked layouts: index [s*32+p, Jg*w + q], b_local = 32*Jg + p
        gt3 = np.stack([gtf[rs], gtp[rs], gte[rs]], -1)          # [BL,3s,3]
        gt3 = gt3.reshape(BLKT, 32, 3, 3).transpose(2, 1, 0, 3)  # [3s,32,J,3]
        rohc = roh[rs].reshape(BLKT, 32, 3, 3).transpose(2, 1, 0, 3)
        mc = m[rs].reshape(BLKT, 32, 3).transpose(2, 1, 0)       # [3s,32,J]
        # pack [gt | roh] f32 and [mi | rohi] u8 per macro-tile block
        gr = np.zeros((3, 32, BLKT * 6), f32)
        mr = np.zeros((3, 32, BLKT * 4), np.uint8)
        for mt in range(nmt):
            js = slice(mt * NBLK, (mt + 1) * NBLK)
            blk = slice(mt * NBLK * 6, mt * NBLK * 6 + NBLK * 3)
            blk2 = slice(mt * NBLK * 6 + NBLK * 3, (mt + 1) * NBLK * 6)
            gr[:, :, blk] = gt3[:, :, js, :].reshape(3, 32, NBLK * 3)
            gr[:, :, blk2] = rohc[:, :, js, :].reshape(3, 32, NBLK * 3)
            mb = slice(mt * NBLK * 4, mt * NBLK * 4 + NBLK)
            mb2 = slice(mt * NBLK * 4 + NBLK, (mt + 1) * NBLK * 4)
            mr[:, :, mb] = mc[:, :, js].astype(np.uint8)
            mr[:, :, mb2] = rohc[:, :, js, :].reshape(
                3, 32, NBLK * 3).astype(np.uint8)
        in_maps.append(dict(
            seq=np.ascontiguousarray(seq[rs].astype(NP_F8).T),
            gr=np.ascontiguousarray(gr.reshape(96, BLKT * 6)),
            mr=np.ascontiguousarray(mr.reshape(96, BLKT * 4)),
            **shared))
    aux = dict(gtf=gtf, gtp=gtp, gte=gte, m=m)
    return in_maps, aux


def assemble(results, aux):
    """Gather per-core outputs; finish the (tiny) loss reductions host-side."""
    f32 = np.float32
    do = np.concatenate([r["dout"] for r in results], 0).astype(f32)
    lpe = np.concatenate([r["lpe"] for r in results], 0).astype(f32)
    df, dp, de = do[:, 0:3], do[:, 3:6], do[:, 6:9]
    lg, pf, pe = lpe[:, 0:3], lpe[:, 3:6], lpe[:, 6:9]
    m, gtf, gtp, gte = aux["m"], aux["gtf"], aux["gtp"], aux["gte"]
    lf = np.sum(np.square(pf - gtf) * m, dtype=np.float64)
    le = np.sum(np.square(pe - gte) * m, dtype=np.float64)
    bce = (np.maximum(lg, 0.0) - lg * gtp
           + np.log1p(np.exp(-np.abs(lg), dtype=np.float64)))
    lp = np.sum(bce * m, dtype=np.float64)
    nm = np.sum(m, dtype=np.float64) + 1e-8
    head = np.array([lf / nm, lp / nm, le / nm], f32)
    return np.concatenate([head, df.ravel(), dp.ravel(), de.ravel()])


_CACHE = {}


def _get_graph(BL):
    if BL not in _CACHE:
        _CACHE[BL] = build_graph(BL)
    return _CACHE[BL]


def _install_profile_hook():
    """Provide antenv.axon_hooks (missing in this image) so trace=True works."""
    import sys, types
    try:
        import antenv.axon_hooks  # noqa: F401
        return
    except ImportError:
        pass
    from trn_agent_boot.trn_boot import _ntff_profile_via_ctypes
    hook = _ntff_profile_via_ctypes('/opt/axon/libaxon_pjrt.so')
    mod = types.ModuleType('antenv.axon_hooks')
    mod._hook = hook
    mod.get_axon_ntff_profile_hook = lambda: mod._hook
    mod.set_axon_ntff_profile_hook = lambda h: setattr(mod, '_hook', h)
    sys.modules['antenv.axon_hooks'] = mod


def run(inputs, trace=False):
    if trace:
        _install_profile_hook()
    BL = inputs["seq_embed"].shape[0] // NCORES
    nc = _get_graph(BL)
    in_maps, aux = prep_inputs(**inputs, BL=BL)
    res = run_bass_kernel_spmd(nc, in_maps, core_ids=list(range(NCORES)),
                               trace=trace)
    out = assemble(res.results, aux)
    return out, res


def kernel(**inputs):
    inputs = {k: np.asarray(v) for k, v in inputs.items()}
    out, _ = run(inputs)
    return out
